# revision 4
# baseline (speedup 1.0000x reference)
"""Trainium2 Bass kernel for nn_CycleNet_EPD (ragged graph edge-phase decoder).

Math (per graph b, with La = edge_len[b], Ba = beta_len[b]):
  ef[e,:4]   = [x[src_e], x[dst_e]]                        (edge features)
  s[beta,:]  = sum_e |SCB[b,beta,e]| * ef[e,:]             (beta < Ba, e < La)
  emb        = relu(s@W1+b1)@W2+b2                         [Ba,64]
  A[beta,:]  = emb@W3a + b3                                [Ba,128]  (W3a=W3[:64])
  G[e,:]     = ef@W3b                                      [La,128]  (W3b=W3[64:])
  H[e,:]     = sum_{beta<Ba} relu(A[beta,:] + |SCB[b,beta,e]|*G[e,:])
               + (64-Ba)*relu(A_pad)          (A_pad = const row for padded beta)
  out[e,:]   = relu((H@W4 + 64*b4 + (64-Ba)*relu(A_pad)@W4)@... )
             = relu(h@W5+b5)@W6+b6 with h = H@W4 + vb
  rows with e >= La are zero.

Device mapping (per graph):
  - stage A: PE transposes |SCB| -> scb_T; small matmuls for s, emb, A, G, K0, vb
  - stage B per (beta, e-chunk): PE K=1 ones-matmul broadcasts |scb[beta, e-slice]|
    across 128 partitions into PSUM; DVE scalar_tensor_tensor computes
    |s2| * G_T (abs is a no-op re-guard); ACT applies relu with per-partition
    bias A_T[:,beta]; PE identity-matmul accumulates the beta-sum in PSUM.
  - out stage per e-chunk: three K=128 matmuls (W4, W5, W6) with ACT
    relu/bias epilogues, PE transpose to [e, 128], DMA to DRAM.

Sharding: per-core work items (graph, e0, e1) fill each core to ~total/8
La*Ba columns, splitting large graphs by edge range (stage A is recomputed on
each core touching a split graph; it is tiny). One NEFF; each core's exact
ragged schedule sits in its own branch of a partition-id If-tree.
Host does only data movement: gather of x rows by edge_index (edge feature
assembly), packing/padding per-core inputs, and scatter of per-core outputs
into the full [B*MAX_E, HID] result (padded rows stay zero).
"""

import sys

sys.path.insert(0, "/opt/trn_rl_repo")

import numpy as np

import concourse.bacc as bacc
import concourse.mybir as mybir
import concourse.tile as tile
from concourse import bass_utils

B, MAX_N, MAX_E, MAX_BETA = 16, 512, 1024, 64
NODE_F, HID = 2, 128
NCORES = 8
F32 = mybir.dt.float32
AF = mybir.ActivationFunctionType
ALU = mybir.AluOpType

ECHUNK = 512  # e-tile for stage B / out stage (one PSUM bank)

LAST_EXEC_NS = None


def _install_ntff_hook():
    """Register the axon NTFF profile hook (image's antenv lacks it)."""
    import types

    try:
        from antenv.axon_hooks import get_axon_ntff_profile_hook  # noqa
        return
    except ImportError:
        pass
    import antenv

    mod = types.ModuleType("antenv.axon_hooks")
    _h = [None]
    mod.set_axon_ntff_profile_hook = lambda h: _h.__setitem__(0, h)
    mod.get_axon_ntff_profile_hook = lambda: _h[0]
    sys.modules["antenv.axon_hooks"] = mod
    antenv.axon_hooks = mod
    if "/root/.axon_site" not in sys.path:
        sys.path.insert(0, "/root/.axon_site")
    from trn_agent_boot.trn_boot import _ntff_profile_via_ctypes

    hook = _ntff_profile_via_ctypes("/opt/axon/libaxon_pjrt.so")
    if hook is not None:
        mod.set_axon_ntff_profile_hook(hook)


def _plan(edge_len, beta_len):
    """Per-core work items (g, e0, e1); large graphs split by edge range."""
    La = [max(1, min(MAX_E, int(v))) for v in edge_len]
    Ba = [max(1, min(MAX_BETA, int(v))) for v in beta_len]
    load = [La[b] * Ba[b] for b in range(B)]
    total = sum(load)
    target = -(-total // NCORES)
    order = sorted(range(B), key=lambda b: -load[b])
    cores = [[] for _ in range(NCORES)]
    c, used = 0, 0
    for g in order:
        e0 = 0
        while e0 < La[g]:
            cap = target - used
            if cap <= 0 and c < NCORES - 1:
                c, used = c + 1, 0
                cap = target
            ne = min(La[g] - e0, max(1, -(-cap // Ba[g])))
            if c == NCORES - 1:
                ne = La[g] - e0
            cores[c].append((g, e0, e0 + ne))
            used += ne * Ba[g]
            e0 += ne
    return La, Ba, cores


def kernel(x, SCB, edge_index, edge_len, beta_len,
           W1, b1, W2, b2, W3, b3, W4, b4, W5, b5, W6, b6):
    x = np.asarray(x, np.float32)
    SCB = np.asarray(SCB, np.float32)
    edge_index = np.asarray(edge_index, np.int32)
    La, Ba, cores = _plan(np.asarray(edge_len), np.asarray(beta_len))
    ngmax = max(len(c) for c in cores)

    # ---- host-side packing (data movement only) ----
    # edge features via index gather
    ef_all = []
    for b in range(B):
        src = edge_index[b, 0, : La[b]]
        dst = edge_index[b, 1, : La[b]]
        ef_all.append(np.concatenate([x[b][src], x[b][dst]], axis=1))  # [La,4]

    scb_off = [[0] * ngmax for _ in range(NCORES)]
    ef_off = [[0] * ngmax for _ in range(NCORES)]
    cmax = 1
    emax = 1
    for c in range(NCORES):
        co = 0
        eo = 0
        for i, (g, e0, e1) in enumerate(cores[c]):
            scb_off[c][i] = co
            ef_off[c][i] = eo
            co += (e1 - e0) * Ba[g]
            eo += La[g]
        cmax = max(cmax, co)
        emax = max(emax, eo)

    in_maps = []
    w_common = {
        "w1": np.ascontiguousarray(W1, np.float32),          # [4,64]
        "w2": np.ascontiguousarray(W2, np.float32),          # [64,64]
        "w3a": np.ascontiguousarray(W3[:64], np.float32),    # [64,128]
        "w3b": np.ascontiguousarray(W3[64:], np.float32),    # [4,128]
        "w4": np.ascontiguousarray(W4, np.float32),
        "w5": np.ascontiguousarray(W5, np.float32),
        "w6": np.ascontiguousarray(W6, np.float32),
        "b1c": np.ascontiguousarray(np.asarray(b1, np.float32)[:, None]),
        "b2c": np.ascontiguousarray(np.asarray(b2, np.float32)[:, None]),
        "b3c": np.ascontiguousarray(np.asarray(b3, np.float32)[:, None]),
        "b4x64": np.ascontiguousarray(64.0 * np.asarray(b4, np.float32)[:, None]),
        "b5c": np.ascontiguousarray(np.asarray(b5, np.float32)[:, None]),
        "b6c": np.ascontiguousarray(np.asarray(b6, np.float32)[:, None]),
        "ones": np.ones((1, 128), np.float32),
        "ident": np.eye(128, dtype=np.float32),
    }
    for c in range(NCORES):
        scb_pack = np.zeros((ngmax * 64, MAX_E), np.float32)
        scbcols = np.zeros((1, cmax), np.float32)
        eft = np.zeros((4, emax), np.float32)
        for i, (g, e0, e1) in enumerate(cores[c]):
            scb_pack[i * 64 : i * 64 + 64, : La[g]] = SCB[g][:, : La[g]]
            scbcols[0, scb_off[c][i] : scb_off[c][i] + (e1 - e0) * Ba[g]] = \
                np.abs(SCB[g][: Ba[g], e0:e1]).reshape(-1)
            eft[:, ef_off[c][i] : ef_off[c][i] + La[g]] = ef_all[g].T
        m = dict(w_common)
        m["scb_pack"] = scb_pack
        m["scbcols"] = scbcols
        m["eft"] = eft
        in_maps.append(m)

    # ---- build program ----
    nc = bacc.Bacc("TRN2", target_bir_lowering=False, debug=False,
                   num_devices=NCORES)
    d_in = {}
    for name, arr in in_maps[0].items():
        d_in[name] = nc.dram_tensor(name, list(arr.shape), F32,
                                    kind="ExternalInput")
    d_out = nc.dram_tensor("out", [ngmax * MAX_E, HID], F32,
                           kind="ExternalOutput")

    with tile.TileContext(nc) as tc:
        pid = nc.partition_id()
        with (
            tc.tile_pool(name="const", bufs=1) as cpool,
            tc.tile_pool(name="sbA", bufs=2) as sbA,
            tc.tile_pool(name="sbB", bufs=3) as sbB,
            tc.tile_pool(name="psS", bufs=2, space="PSUM") as psS,
            tc.tile_pool(name="psH", bufs=2, space="PSUM") as psH,
            tc.tile_pool(name="psO", bufs=2, space="PSUM") as psO,
            tc.tile_pool(name="psM", bufs=2, space="PSUM") as psM,
        ):
            cst = {}
            for name, shape in [
                ("w1", [4, 64]), ("w2", [64, 64]), ("w3a", [64, 128]),
                ("w3b", [4, 128]), ("w4", [128, 128]), ("w5", [128, 128]),
                ("w6", [128, 128]), ("b1c", [64, 1]), ("b2c", [64, 1]),
                ("b3c", [128, 1]), ("b4x64", [128, 1]), ("b5c", [128, 1]),
                ("b6c", [128, 1]), ("ones", [1, 128]), ("ident", [128, 128]),
            ]:
                t = cpool.tile(shape, F32, tag=name)
                nc.sync.dma_start(t[:], d_in[name].ap())
                cst[name] = t

            def build_graph(c, slot, g, es, ee):
                la, ba = La[g], Ba[g]
                lw = ee - es  # width of this core's edge slice
                nech = (la + 127) // 128  # 128-e chunks for transposes / s
                goff_s = scb_off[c][slot]
                goff_e = ef_off[c][slot]

                # ---- stage A ----
                scb_sb = sbA.tile([64, MAX_E], F32, tag="scb")
                nc.sync.dma_start(
                    scb_sb[:, :la],
                    d_in["scb_pack"].ap()[slot * 64 : slot * 64 + 64, :la],
                )
                # |SCB| on the rows used for s (in-place, ACT Abs)
                nc.scalar.activation(scb_sb[:ba, :la], scb_sb[:ba, :la],
                                     AF.Abs, bias=0.0, scale=1.0)
                eft_sb = sbA.tile([4, MAX_E], F32, tag="eft")
                nc.sync.dma_start(
                    eft_sb[:, :la], d_in["eft"].ap()[:, goff_e : goff_e + la]
                )
                # scb_T chunks [128e, 64b] via PE transpose
                scbT_sb = sbA.tile([128, 64 * 8], F32, tag="scbT")
                for ec in range(nech):
                    n = min(128, la - ec * 128)
                    tp = psM.tile([128, 64], F32, tag="pM")
                    nc.tensor.transpose(
                        tp[:n, :], scb_sb[:, ec * 128 : ec * 128 + n],
                        cst["ident"][:64, :64],
                    )
                    nc.scalar.copy(scbT_sb[:n, ec * 64 : ec * 64 + 64], tp[:n, :])
                # ef rows [e,4] per 128-chunk (for s); from eft via transpose
                efr_sb = sbA.tile([128, 4 * 8], F32, tag="efr")
                for ec in range(nech):
                    n = min(128, la - ec * 128)
                    tp2 = psM.tile([128, 4], F32, tag="pM")
                    nc.tensor.transpose(
                        tp2[:n, :], eft_sb[:, ec * 128 : ec * 128 + n],
                        cst["ident"][:4, :4],
                    )
                    nc.scalar.copy(efr_sb[:n, ec * 4 : ec * 4 + 4], tp2[:n, :])
                # G_T[h, e] = sum_k W3b[k,h] ef_T[k,e]
                GT_sb = sbA.tile([128, MAX_E], F32, tag="GT")
                for e0 in range(es, ee, ECHUNK):
                    n = min(ECHUNK, ee - e0)
                    pg = psM.tile([128, ECHUNK], F32, tag="pM")
                    nc.tensor.matmul(pg[:, :n], cst["w3b"][:],
                                     eft_sb[:, e0 : e0 + n],
                                     start=True, stop=True)
                    nc.scalar.copy(GT_sb[:, e0 : e0 + n], pg[:, :n])
                # s_T[k, beta] = sum_e ef[e,k] |scb|_T[e, beta]
                ps_s = psM.tile([4, 128], F32, tag="pM")
                for ec in range(nech):
                    n = min(128, la - ec * 128)
                    nc.tensor.matmul(
                        ps_s[:, :ba],
                        efr_sb[:n, ec * 4 : ec * 4 + 4],
                        scbT_sb[:n, ec * 64 : ec * 64 + ba],
                        start=(ec == 0), stop=(ec == nech - 1),
                    )
                s_sb = sbA.tile([4, 65], F32, tag="s")
                nc.vector.memset(s_sb[:], 0.0)
                nc.scalar.copy(s_sb[:, :ba], ps_s[:, :ba])
                # emb / A chain (one padded col at index ba -> A_pad)
                nb = ba + 1
                pe1 = psM.tile([64, 65], F32, tag="pM")
                nc.tensor.matmul(pe1[:, :nb], cst["w1"][:], s_sb[:, :nb],
                                 start=True, stop=True)
                e1_sb = sbA.tile([64, 65], F32, tag="e1")
                nc.scalar.activation(e1_sb[:, :nb], pe1[:, :nb], AF.Relu,
                                     bias=cst["b1c"][:], scale=1.0)
                pe2 = psM.tile([64, 65], F32, tag="pM")
                nc.tensor.matmul(pe2[:, :nb], cst["w2"][:], e1_sb[:, :nb],
                                 start=True, stop=True)
                e2_sb = sbA.tile([64, 65], F32, tag="e2")
                nc.scalar.activation(e2_sb[:, :nb], pe2[:, :nb], AF.Identity,
                                     bias=cst["b2c"][:], scale=1.0)
                pa = psM.tile([128, 65], F32, tag="pM")
                nc.tensor.matmul(pa[:, :nb], cst["w3a"][:], e2_sb[:, :nb],
                                 start=True, stop=True)
                A_sb = sbA.tile([128, 65], F32, tag="A")
                nc.scalar.activation(A_sb[:, :nb], pa[:, :nb], AF.Identity,
                                     bias=cst["b3c"][:], scale=1.0)
                # K0 = relu(A_pad); vb = (64-Ba) * K0@W4 + 64*b4
                K0_sb = sbA.tile([128, 1], F32, tag="K0")
                nc.scalar.activation(K0_sb[:], A_sb[:, ba : ba + 1], AF.Relu,
                                     bias=0.0, scale=1.0)
                pk = psM.tile([128, 1], F32, tag="pM")
                nc.tensor.matmul(pk[:], cst["w4"][:], K0_sb[:],
                                 start=True, stop=True)
                vb_sb = sbA.tile([128, 1], F32, tag="vb")
                nc.scalar.activation(vb_sb[:], pk[:], AF.Identity,
                                     bias=cst["b4x64"][:],
                                     scale=float(64 - ba))

                # ---- stage B + out stage, per 512-e chunk ----
                for e0 in range(es, ee, ECHUNK):
                    n = min(ECHUNK, ee - e0)
                    pH = psH.tile([128, ECHUNK], F32, tag="H")
                    for bi in range(ba):
                        co = goff_s + bi * lw + (e0 - es)
                        srow = sbB.tile([1, ECHUNK], F32, tag="srow")
                        nc.sync.dma_start(
                            srow[:, :n], d_in["scbcols"].ap()[:, co : co + n]
                        )
                        ps2 = psS.tile([128, ECHUNK], F32, tag="s2")
                        nc.tensor.matmul(ps2[:, :n], cst["ones"][:],
                                         srow[:, :n], start=True, stop=True)
                        t_sb = sbB.tile([128, ECHUNK], F32, tag="t")
                        nc.vector.tensor_mul(
                            t_sb[:, :n], ps2[:, :n], GT_sb[:, e0 : e0 + n]
                        )
                        r_sb = sbB.tile([128, ECHUNK], F32, tag="r")
                        nc.scalar.activation(r_sb[:, :n], t_sb[:, :n], AF.Relu,
                                             bias=A_sb[:, bi : bi + 1],
                                             scale=1.0)
                        nc.tensor.matmul(pH[:, :n], cst["ident"][:],
                                         r_sb[:, :n], start=(bi == 0),
                                         stop=(bi == ba - 1))
                    H_sb = sbB.tile([128, ECHUNK], F32, tag="Hs")
                    nc.scalar.copy(H_sb[:, :n], pH[:, :n])
                    p1 = psO.tile([128, ECHUNK], F32, tag="pO")
                    nc.tensor.matmul(p1[:, :n], cst["w4"][:], H_sb[:, :n],
                                     start=True, stop=True)
                    r5 = sbB.tile([128, ECHUNK], F32, tag="r5")
                    nc.scalar.activation(r5[:, :n], p1[:, :n], AF.Identity,
                                         bias=vb_sb[:], scale=1.0)
                    p2 = psO.tile([128, ECHUNK], F32, tag="pO")
                    nc.tensor.matmul(p2[:, :n], cst["w5"][:], r5[:, :n],
                                     start=True, stop=True)
                    r6 = sbB.tile([128, ECHUNK], F32, tag="r6")
                    nc.scalar.activation(r6[:, :n], p2[:, :n], AF.Relu,
                                         bias=cst["b5c"][:], scale=1.0)
                    p3 = psO.tile([128, ECHUNK], F32, tag="pO")
                    nc.tensor.matmul(p3[:, :n], cst["w6"][:], r6[:, :n],
                                     start=True, stop=True)
                    o_sb = sbB.tile([128, ECHUNK], F32, tag="o")
                    nc.scalar.activation(o_sb[:, :n], p3[:, :n], AF.Identity,
                                         bias=cst["b6c"][:], scale=1.0)
                    for et in range(0, n, 128):
                        m = min(128, n - et)
                        po = psM.tile([128, 128], F32, tag="pM")
                        nc.tensor.transpose(po[:m, :], o_sb[:, et : et + m],
                                            cst["ident"][:])
                        oT = sbB.tile([128, 128], F32, tag="oT")
                        nc.scalar.copy(oT[:m, :], po[:m, :])
                        r0 = slot * MAX_E + e0 + et
                        nc.sync.dma_start(d_out.ap()[r0 : r0 + m, :],
                                          oT[:m, :])

            def build_core(c):
                for slot, (g, e0, e1) in enumerate(cores[c]):
                    build_graph(c, slot, g, e0, e1)

            def dispatch(lo, hi):
                if hi - lo == 1:
                    build_core(lo)
                    return
                mid = (lo + hi) // 2
                with tc.If(pid < mid) as cmp:
                    dispatch(lo, mid)
                with cmp.Else():
                    dispatch(mid, hi)

            dispatch(0, NCORES)

    import os
    if os.environ.get("KERNEL_BUILD_ONLY"):
        return np.zeros((B * MAX_E, HID), np.float32)
    nc.compile()
    if os.environ.get("KERNEL_COMPILE_ONLY"):
        import tempfile
        neff = bass_utils.compile_bass_kernel(nc, tempfile.mkdtemp())
        print("NEFF:", neff)
        return np.zeros((B * MAX_E, HID), np.float32)
    run_kwargs = {}
    if os.environ.get("KERNEL_TRACE"):
        _install_ntff_hook()
        tdir = os.environ.get("KERNEL_TRACE_DIR") or "/tmp/ktrace"
        os.makedirs(tdir, exist_ok=True)
        run_kwargs = dict(
            trace=True,
            trace_cores=list(range(NCORES)),
            tmpdir=tdir,
        )
    res = bass_utils.run_bass_kernel_spmd(
        nc, in_maps, core_ids=list(range(NCORES)), **run_kwargs
    )
    global LAST_EXEC_NS
    LAST_EXEC_NS = res.exec_time_ns

    out = np.zeros((B * MAX_E, HID), np.float32)
    for c in range(NCORES):
        oc = res.results[c]["out"]
        for slot, (g, e0, e1) in enumerate(cores[c]):
            out[g * MAX_E + e0 : g * MAX_E + e1] = \
                oc[slot * MAX_E + e0 : slot * MAX_E + e1]
    return out



# revision 21
# speedup vs baseline: 2.7406x; 2.7406x over previous
"""Trainium2 Bass kernel for nn_CycleNet_EPD (ragged graph edge-phase decoder).

Math (per graph b, with La = edge_len[b], Ba = beta_len[b]):
  ef[e,:4]   = [x[src_e], x[dst_e]]                        (edge features)
  s[beta,:]  = sum_e |SCB[b,beta,e]| * ef[e,:]             (beta < Ba, e < La)
  emb        = relu(s@W1+b1)@W2+b2                         [Ba,64]
  A[beta,:]  = emb@W3a + b3                                [Ba,128]  (W3a=W3[:64])
  G[e,:]     = ef@W3b                                      [La,128]  (W3b=W3[64:])
  H[e,:]     = sum_{beta<Ba} relu(A[beta,:] + |SCB[b,beta,e]|*G[e,:])
  out[e,:]   = relu((H@W4 + vb)@W5 + b5)@W6 + b6
               vb = 64*b4 + (64-Ba)*relu(A_pad)@W4  (pad-beta contribution)
  rows with e >= La are zero.

Device mapping (v2 — PE removed from the inner loop):
  - |scb[beta,e]|*G[h,e] == W3b^T @ (eft * scb_row), so per (beta, e-chunk):
      DVE  : efs = eft16 * scb16_row      [4, n] fp16 (4x mode)
      PE   : ps_t = w3b16^T @ efs         [128, n] PSUM (K=4, fp16 1cy/col)
      relu : one of three parallel paths (balances ACT/DVE/Pool):
        'A': ACT relu(ps_t + A[:,beta]) -> r fp16; PE ident-matmul accumulates
             the beta-sum in PSUM (fp16 1cy/col)
        'D': DVE STT (ps_t + A) max 0 -> t fp16; DVE adds into accD fp32 SBUF
        'P': same on GpSimd into accP
  - s comes from host-packed transposed chunks (scbT16/efr16) via tiny fp16
    matmuls; emb/A/vb chain in fp32 (small).
  - out stage: H = pH + accD + accP; fp16 matmuls W4/W5; final 128-col chunks
    use lhsT=r6-chunk so the output lands as [e,128] rows; b6 is preloaded
    into PSUM with a 1-row ones-matmul (start=True), W6 matmul accumulates.

Sharding: per-core work items (graph, e0, e1) fill each core to ~total/8 of
La*(Ba+4) load, splitting large graphs by edge range (stage A is recomputed
per core from full-range host-packed data). One NEFF; each core's ragged
schedule sits in its own branch of a partition-id If-tree. Host does only
data movement: gather of x rows by edge_index, abs/cast/transpose packing,
and scatter of per-core outputs.
"""

import sys

sys.path.insert(0, "/opt/trn_rl_repo")

import numpy as np

import concourse.bacc as bacc
import concourse.mybir as mybir
import concourse.tile as tile
from concourse import bass_utils

B, MAX_N, MAX_E, MAX_BETA = 16, 512, 1024, 64
NODE_F, HID = 2, 128
NCORES = 8
F32 = mybir.dt.float32
F16 = mybir.dt.float16
AF = mybir.ActivationFunctionType
ALU = mybir.AluOpType

ECHUNK = 512  # e-tile for stage B / out stage (one PSUM bank)
NCH = MAX_E // 128  # 128-e chunks per graph (host-packed transposes)

# relu-path pattern per beta index: A=ACT relu + PE accum,
# D=DVE STT relu + GpSimd accum (GpSimd cannot read PSUM)
import os
PATTERN = os.environ.get("KERNEL_PAT", "AAD")

LAST_EXEC_NS = None


def _install_ntff_hook():
    """Register the axon NTFF profile hook (image's antenv lacks it)."""
    import types

    try:
        from antenv.axon_hooks import get_axon_ntff_profile_hook  # noqa
        return
    except ImportError:
        pass
    import antenv

    mod = types.ModuleType("antenv.axon_hooks")
    _h = [None]
    mod.set_axon_ntff_profile_hook = lambda h: _h.__setitem__(0, h)
    mod.get_axon_ntff_profile_hook = lambda: _h[0]
    sys.modules["antenv.axon_hooks"] = mod
    antenv.axon_hooks = mod
    if "/root/.axon_site" not in sys.path:
        sys.path.insert(0, "/root/.axon_site")
    from trn_agent_boot.trn_boot import _ntff_profile_via_ctypes

    hook = _ntff_profile_via_ctypes("/opt/axon/libaxon_pjrt.so")
    if hook is not None:
        mod.set_axon_ntff_profile_hook(hook)


def _plan(edge_len, beta_len):
    """Per-core work items (g, e0, e1); large graphs split by edge range."""
    La = [max(1, min(MAX_E, int(v))) for v in edge_len]
    Ba = [max(1, min(MAX_BETA, int(v))) for v in beta_len]
    load = [La[b] * (Ba[b] + 4) for b in range(B)]
    total = sum(load)
    target = -(-total // NCORES)
    order = sorted(range(B), key=lambda b: -load[b])
    cores = [[] for _ in range(NCORES)]
    c, used = 0, 0
    for g in order:
        e0 = 0
        while e0 < La[g]:
            cap = target - used
            if cap <= 0 and c < NCORES - 1:
                c, used = c + 1, 0
                cap = target
            ne = min(La[g] - e0, max(1, -(-cap // (Ba[g] + 4))))
            if c == NCORES - 1:
                ne = La[g] - e0
            cores[c].append((g, e0, e0 + ne))
            used += ne * (Ba[g] + 4)
            e0 += ne
    return La, Ba, cores


def kernel(x, SCB, edge_index, edge_len, beta_len,
           W1, b1, W2, b2, W3, b3, W4, b4, W5, b5, W6, b6):
    x = np.asarray(x, np.float32)
    SCB = np.asarray(SCB, np.float32)
    edge_index = np.asarray(edge_index, np.int32)
    La, Ba, cores = _plan(np.asarray(edge_len), np.asarray(beta_len))
    ngmax = max(len(c) for c in cores)

    # ---- host-side packing (data movement only) ----
    SCBa = np.abs(SCB)
    # edge features via index gather
    ef_all = []
    for b in range(B):
        src = edge_index[b, 0, : La[b]]
        dst = edge_index[b, 1, : La[b]]
        ef_all.append(np.concatenate([x[b][src], x[b][dst]], axis=1))  # [La,4]

    scb_off = [[0] * ngmax for _ in range(NCORES)]
    ef_off = [[0] * ngmax for _ in range(NCORES)]
    cmax = 1
    emax = 1
    for c in range(NCORES):
        co = 0
        eo = 0
        for i, (g, e0, e1) in enumerate(cores[c]):
            scb_off[c][i] = co
            ef_off[c][i] = eo
            co += (e1 - e0) * Ba[g]
            eo += e1 - e0
        cmax = max(cmax, co)
        emax = max(emax, eo)

    w_common = {
        "w1": np.ascontiguousarray(W1, np.float32),          # [4,64]
        "w2": np.ascontiguousarray(W2, np.float32),          # [64,64]
        "w3a": np.ascontiguousarray(W3[:64], np.float32),    # [64,128]
        "w3b16": np.ascontiguousarray(W3[64:], np.float16),  # [4,128]
        "w4_16": np.ascontiguousarray(W4, np.float16),
        "w5_16": np.ascontiguousarray(W5, np.float16),
        "w6_16": np.ascontiguousarray(W6, np.float16),
        "b1c": np.ascontiguousarray(np.asarray(b1, np.float32)[:, None]),
        "b2c": np.ascontiguousarray(np.asarray(b2, np.float32)[:, None]),
        "b3c": np.ascontiguousarray(np.asarray(b3, np.float32)[:, None]),
        "b4x64": np.ascontiguousarray(64.0 * np.asarray(b4, np.float32)[:, None]),
        "b5c": np.ascontiguousarray(np.asarray(b5, np.float32)[:, None]),
        "b6r16": np.ascontiguousarray(np.asarray(b6, np.float16)[None, :]),
        "ones16": np.ones((1, 128), np.float16),
        "ident16": np.eye(128, dtype=np.float16),
    }
    in_maps = []
    for c in range(NCORES):
        # |SCB|^T in 128-e chunks: [128, slot*NCH*64 + ec*64 + beta]
        scbT16 = np.zeros((128, ngmax * NCH * 64), np.float16)
        # ef rows in 128-e chunks: [128, slot*NCH*4 + ec*4 + k]
        efr16 = np.zeros((128, ngmax * NCH * 4), np.float16)
        # |SCB| beta-major per item slice, replicated to 4 partitions
        scb16 = np.zeros((4, cmax), np.float16)
        # ef^T per item slice
        eft16 = np.zeros((4, emax), np.float16)
        for i, (g, e0, e1) in enumerate(cores[c]):
            la, ba = La[g], Ba[g]
            nech = (la + 127) // 128
            for ec in range(nech):
                n = min(128, la - ec * 128)
                base = (i * NCH + ec) * 64
                scbT16[:n, base : base + 64] = \
                    SCBa[g][:, ec * 128 : ec * 128 + n].T
                baseE = (i * NCH + ec) * 4
                efr16[:n, baseE : baseE + 4] = \
                    ef_all[g][ec * 128 : ec * 128 + n, :]
            so = scb_off[c][i]
            scb16[:, so : so + ba * (e1 - e0)] = \
                SCBa[g][:ba, e0:e1].reshape(-1)[None, :]
            eo = ef_off[c][i]
            eft16[:, eo : eo + (e1 - e0)] = ef_all[g][e0:e1, :].T
        m = dict(w_common)
        m["scbT16"] = scbT16
        m["efr16"] = efr16
        m["scb16"] = scb16
        m["eft16"] = eft16
        in_maps.append(m)

    # ---- build program ----
    nc = bacc.Bacc("TRN2", target_bir_lowering=False, debug=False,
                   num_devices=NCORES)
    d_in = {}
    for name, arr in in_maps[0].items():
        dt = F16 if arr.dtype == np.float16 else F32
        d_in[name] = nc.dram_tensor(name, list(arr.shape), dt,
                                    kind="ExternalInput")
    d_out = nc.dram_tensor("out", [ngmax * MAX_E, HID], F32,
                           kind="ExternalOutput")
    DBG = bool(os.environ.get("KERNEL_DEBUG"))
    d_dbg = {}
    if DBG:
        for nm, shape, dt in [
            ("dbg_s", [4, 65], F32), ("dbg_A", [128, 65], F32),
            ("dbg_vb", [128, 1], F32), ("dbg_efs", [4, ECHUNK], F16),
            ("dbg_r", [128, ECHUNK], F16), ("dbg_H", [128, ECHUNK], F16),
            ("dbg_r5", [128, ECHUNK], F16), ("dbg_r6", [128, ECHUNK], F16),
            ("dbg_o", [128, ECHUNK], F32),
        ]:
            d_dbg[nm] = nc.dram_tensor(nm, shape, dt, kind="ExternalOutput")

    with tile.TileContext(nc) as tc:
        pid = nc.partition_id()
        with (
            tc.tile_pool(name="const", bufs=1) as cpool,
            tc.tile_pool(name="sbA", bufs=2) as sbA,
            tc.tile_pool(name="sbB", bufs=3) as sbB,
            tc.tile_pool(name="sbC", bufs=2) as sbC,
            tc.tile_pool(name="psS", bufs=1, space="PSUM") as psS,
            tc.tile_pool(name="psT", bufs=3, space="PSUM") as psT,
            tc.tile_pool(name="psH", bufs=2, space="PSUM") as psH,
            tc.tile_pool(name="psO", bufs=2, space="PSUM") as psO,
        ):
            # per-core packed inputs, loaded once (gpsimd queue: cheap issue)
            big = {}
            for name in ("scbT16", "efr16", "scb16", "eft16"):
                t = cpool.tile(list(in_maps[0][name].shape), F16, tag=name)
                nc.gpsimd.dma_start(t[:], d_in[name].ap())
                big[name] = t
            cst = {}
            for name, arr in w_common.items():
                dt = F16 if arr.dtype == np.float16 else F32
                t = cpool.tile(list(arr.shape), dt, tag=name)
                nc.gpsimd.dma_start(t[:], d_in[name].ap())
                cst[name] = t
            zeros16 = cpool.tile([128, ECHUNK], F16, tag="zeros16")
            nc.vector.memset(zeros16[:], 0.0)

            def build_graph(c, slot, g, es, ee):
                la, ba = La[g], Ba[g]
                nech = (la + 127) // 128
                goff_s = scb_off[c][slot]
                goff_e = ef_off[c][slot]
                lw = ee - es

                # ---- stage A: s -> emb -> A, vb (from host-packed chunks) --
                ps_s = psS.tile([4, 128], F32, tag="pS")
                for ec in range(nech):
                    n = min(128, la - ec * 128)
                    base = (slot * NCH + ec) * 64
                    baseE = (slot * NCH + ec) * 4
                    nc.tensor.matmul(
                        ps_s[:, :ba],
                        big["efr16"][:n, baseE : baseE + 4],
                        big["scbT16"][:n, base : base + ba],
                        start=(ec == 0), stop=(ec == nech - 1),
                    )
                s_sb = sbA.tile([4, 65], F32, tag="s")
                nc.vector.memset(s_sb[:], 0.0)
                nc.vector.tensor_copy(s_sb[:, :ba], ps_s[:, :ba])
                nb = ba + 1
                pe1 = psS.tile([64, 65], F32, tag="pS")
                nc.tensor.matmul(pe1[:, :nb], cst["w1"][:], s_sb[:, :nb],
                                 start=True, stop=True)
                e1_sb = sbA.tile([64, 65], F32, tag="e1")
                nc.scalar.activation(e1_sb[:, :nb], pe1[:, :nb], AF.Relu,
                                     bias=cst["b1c"][:], scale=1.0)
                pe2 = psS.tile([64, 65], F32, tag="pS")
                nc.tensor.matmul(pe2[:, :nb], cst["w2"][:], e1_sb[:, :nb],
                                 start=True, stop=True)
                e2_sb = sbA.tile([64, 65], F32, tag="e2")
                nc.scalar.activation(e2_sb[:, :nb], pe2[:, :nb], AF.Identity,
                                     bias=cst["b2c"][:], scale=1.0)
                pa = psS.tile([128, 65], F32, tag="pS")
                nc.tensor.matmul(pa[:, :nb], cst["w3a"][:], e2_sb[:, :nb],
                                 start=True, stop=True)
                A_sb = sbA.tile([128, 65], F32, tag="A")
                nc.scalar.activation(A_sb[:, :nb], pa[:, :nb], AF.Identity,
                                     bias=cst["b3c"][:], scale=1.0)
                # K0 = relu(A_pad); vb = (64-Ba)*K0@W4 + 64*b4
                K0_sb = sbA.tile([128, 1], F16, tag="K0")
                nc.scalar.activation(K0_sb[:], A_sb[:, ba : ba + 1], AF.Relu,
                                     bias=0.0, scale=1.0)
                pk = psS.tile([128, 1], F32, tag="pS")
                nc.tensor.matmul(pk[:], cst["w4_16"][:], K0_sb[:],
                                 start=True, stop=True)
                vb_sb = sbA.tile([128, 1], F32, tag="vb")
                nc.scalar.activation(vb_sb[:], pk[:], AF.Identity,
                                     bias=cst["b4x64"][:],
                                     scale=float(64 - ba))
                if DBG and slot == 0:
                    nc.sync.dma_start(d_dbg["dbg_s"].ap(), s_sb[:])
                    nc.sync.dma_start(d_dbg["dbg_A"].ap(), A_sb[:])
                    nc.sync.dma_start(d_dbg["dbg_vb"].ap(), vb_sb[:])

                # ---- stage B + out stage, per 512-e chunk ----
                paths = [(PATTERN[bi % len(PATTERN)] if bi else "A")
                         for bi in range(ba)]
                a_betas = [bi for bi in range(ba) if paths[bi] == "A"]
                d_betas = [bi for bi in range(ba) if paths[bi] == "D"]

                for e0 in range(es, ee, ECHUNK):
                    n = min(ECHUNK, ee - e0)
                    pH = psH.tile([128, ECHUNK], F32, tag="H")
                    accD = None
                    if d_betas:
                        accD = sbB.tile([128, ECHUNK], F32, tag="accD",
                                        name="accD")

                    # software pipeline: stage ps_t two betas ahead
                    pst = {}

                    dbg_here = DBG and slot == 0 and e0 == es

                    def emit_mm(bi):
                        co = goff_s + bi * lw + (e0 - es)
                        efs = sbB.tile([4, ECHUNK], F16, tag="efs")
                        nc.vector.tensor_mul(
                            efs[:, :n], big["eft16"][:, goff_e + (e0 - es) :
                                                     goff_e + (e0 - es) + n],
                            big["scb16"][:, co : co + n])
                        if dbg_here and bi == 0:
                            nc.sync.dma_start(d_dbg["dbg_efs"].ap()[:, :n],
                                              efs[:, :n])
                        ps_t = psT.tile([128, ECHUNK], F32, tag="T")
                        nc.tensor.matmul(ps_t[:, :n], cst["w3b16"][:],
                                         efs[:, :n], start=True, stop=True)
                        pst[bi] = ps_t

                    def emit_relu(bi, first_a):
                        ps_t = pst.pop(bi)
                        A_col = A_sb[:, bi : bi + 1]
                        if paths[bi] == "A":
                            r_sb = sbB.tile([128, ECHUNK], F16, tag="r")
                            nc.scalar.activation(r_sb[:, :n], ps_t[:, :n],
                                                 AF.Relu, bias=A_col,
                                                 scale=1.0)
                            if dbg_here and bi == 0:
                                nc.sync.dma_start(
                                    d_dbg["dbg_r"].ap()[:, :n], r_sb[:, :n])
                            nc.tensor.matmul(pH[:, :n], cst["ident16"][:],
                                             r_sb[:, :n],
                                             start=first_a,
                                             stop=(bi == a_betas[-1]))
                        else:
                            if bi == d_betas[0]:
                                nc.vector.scalar_tensor_tensor(
                                    accD[:, :n], ps_t[:, :n], A_col,
                                    zeros16[:, :n], ALU.add, ALU.max)
                            else:
                                t2 = sbB.tile([128, ECHUNK], F16, tag="t2")
                                nc.vector.scalar_tensor_tensor(
                                    t2[:, :n], ps_t[:, :n], A_col,
                                    zeros16[:, :n], ALU.add, ALU.max)
                                nc.gpsimd.tensor_tensor(
                                    accD[:, :n], accD[:, :n], t2[:, :n],
                                    ALU.add)

                    LOOKAHEAD = 2
                    for k in range(min(LOOKAHEAD, ba)):
                        emit_mm(k)
                    for bi in range(ba):
                        emit_relu(bi, first_a=(bi == a_betas[0]))
                        if bi + LOOKAHEAD < ba:
                            emit_mm(bi + LOOKAHEAD)

                    # merge partial sums -> H16
                    H16 = sbC.tile([128, ECHUNK], F16, tag="H16")
                    if accD is not None:
                        nc.vector.tensor_tensor(H16[:, :n], pH[:, :n],
                                                accD[:, :n], ALU.add)
                    else:
                        nc.vector.tensor_copy(H16[:, :n], pH[:, :n])

                    if dbg_here:
                        nc.sync.dma_start(d_dbg["dbg_H"].ap()[:, :n],
                                          H16[:, :n])
                    # out stage
                    p1 = psO.tile([128, ECHUNK], F32, tag="pO")
                    nc.tensor.matmul(p1[:, :n], cst["w4_16"][:], H16[:, :n],
                                     start=True, stop=True)
                    r5 = sbC.tile([128, ECHUNK], F16, tag="r5")
                    nc.scalar.activation(r5[:, :n], p1[:, :n], AF.Identity,
                                         bias=vb_sb[:], scale=1.0)
                    p2 = psO.tile([128, ECHUNK], F32, tag="pO")
                    nc.tensor.matmul(p2[:, :n], cst["w5_16"][:], r5[:, :n],
                                     start=True, stop=True)
                    r6 = sbC.tile([128, ECHUNK], F16, tag="r6")
                    nc.scalar.activation(r6[:, :n], p2[:, :n], AF.Relu,
                                         bias=cst["b5c"][:], scale=1.0)
                    # final: [e,128] rows via lhsT=r6 chunks, b6 preloaded
                    pF = psO.tile([128, ECHUNK], F32, tag="pO")
                    o_sb = sbC.tile([128, ECHUNK], F32, tag="o")
                    for et in range(0, n, 128):
                        m = min(128, n - et)
                        nc.tensor.matmul(pF[:m, et : et + 128],
                                         cst["ones16"][:, :m],
                                         cst["b6r16"][:],
                                         start=True, stop=False)
                        nc.tensor.matmul(pF[:m, et : et + 128],
                                         r6[:, et : et + m],
                                         cst["w6_16"][:],
                                         start=False, stop=True)
                    # pF free dim is (e-chunk, j): cover all j-cols of every
                    # live chunk, not the first n columns
                    wn = ((n + 127) // 128) * 128
                    nc.vector.tensor_copy(o_sb[:, :wn], pF[:, :wn])
                    if dbg_here:
                        nc.sync.dma_start(d_dbg["dbg_r5"].ap()[:, :n],
                                          r5[:, :n])
                        nc.sync.dma_start(d_dbg["dbg_r6"].ap()[:, :n],
                                          r6[:, :n])
                        nc.sync.dma_start(d_dbg["dbg_o"].ap()[:, :n],
                                          o_sb[:, :n])
                    for et in range(0, n, 128):
                        m = min(128, n - et)
                        r0 = slot * MAX_E + e0 + et
                        nc.sync.dma_start(d_out.ap()[r0 : r0 + m, :],
                                          o_sb[:m, et : et + 128])

            def build_core(c):
                for slot, (g, e0, e1) in enumerate(cores[c]):
                    build_graph(c, slot, g, e0, e1)

            def dispatch(lo, hi):
                if hi - lo == 1:
                    build_core(lo)
                    return
                mid = (lo + hi) // 2
                with tc.If(pid < mid) as cmp:
                    dispatch(lo, mid)
                with cmp.Else():
                    dispatch(mid, hi)

            dispatch(0, NCORES)

    if os.environ.get("KERNEL_BUILD_ONLY"):
        return np.zeros((B * MAX_E, HID), np.float32)
    nc.compile()
    if os.environ.get("KERNEL_COMPILE_ONLY"):
        import tempfile
        neff = bass_utils.compile_bass_kernel(nc, tempfile.mkdtemp())
        print("NEFF:", neff)
        return np.zeros((B * MAX_E, HID), np.float32)
    run_kwargs = {}
    if os.environ.get("KERNEL_TRACE"):
        _install_ntff_hook()
        tdir = os.environ.get("KERNEL_TRACE_DIR") or "/tmp/ktrace"
        os.makedirs(tdir, exist_ok=True)
        run_kwargs = dict(
            trace=True,
            trace_cores=list(range(NCORES)),
            tmpdir=tdir,
        )
    res = bass_utils.run_bass_kernel_spmd(
        nc, in_maps, core_ids=list(range(NCORES)), **run_kwargs
    )
    global LAST_EXEC_NS, LAST_RESULTS, LAST_PLAN
    LAST_EXEC_NS = res.exec_time_ns
    LAST_RESULTS = res.results
    LAST_PLAN = (La, Ba, cores)

    out = np.zeros((B * MAX_E, HID), np.float32)
    for c in range(NCORES):
        oc = res.results[c]["out"]
        for slot, (g, e0, e1) in enumerate(cores[c]):
            out[g * MAX_E + e0 : g * MAX_E + e1] = \
                oc[slot * MAX_E + e0 : slot * MAX_E + e1]
    return out
